# revision 20
# baseline (speedup 1.0000x reference)
"""NormMixAttention Trainium2 kernel — 8-core SPMD, sequence-sharded.

Strategy: shard the 4096-token sequence into 8 segments of 512 tokens (all 4
batches on every core; 2048 token-columns per core). Activations live
feature-major on chip ([d on partitions, tokens on free]); the host
pre-transposes query and the weights so no on-chip input transpose is needed.

Two SPMD launches (the linear branch's inter-chunk kv recurrence crosses
segment boundaries):
  launch 1: k_lin (gelu) + v_lin projections, per-chunk kv outer products.
  host:     exclusive prefix-sum of the 64 per-chunk kv states (tiny).
  launch 2: everything else (q_lin, local branch, intra+inter attention,
            layernorm / gated rmsnorm, fused out-projections).

Matmul dtypes: projections in float32r (full PE rate, tf32-like precision),
attention-block and out-proj matmuls in fp16 (small free dims), PSUM fp32.
LN gamma/beta, grn scale and the 0.5 branch-average are folded into the
out-projection weights/bias on the host (exact algebra).
"""
import numpy as np

import concourse.bass as bass
import concourse.mybir as mybir
import concourse.tile as tile
from concourse import bacc
from concourse.bass_utils import run_bass_kernel_spmd

F32, F32R, F16 = mybir.dt.float32, mybir.dt.float32r, mybir.dt.float16
AF = mybir.ActivationFunctionType
from concourse.alu_op_type import AluOpType as ALU

N, B, E, H, C = 4096, 4, 1024, 16, 64
D = E // 2            # 512
DH = D // H           # 32
NCORES = 8
SEG = N // NCORES     # 512 tokens/core
LSEG = SEG // C       # 8 chunks/core/batch
R = B * SEG           # 2048 columns/core
RT = 256              # r-tile width
NRT = R // RT         # 8 r-tiles/core

_CACHE = {}
TRACE = False
LAST_EXEC_NS = []


def _mm(nc, out, lhsT, rhs, start, stop, tp=None, skip=False):
    if tp is not None and tp == (0, 0):
        tp = None
    nc.tensor.matmul(out, lhsT, rhs, start=start, stop=stop, tile_position=tp,
                     skip_group_check=skip)


def _build_launch1():
    nc = bacc.Bacc("TRN2", target_bir_lowering=False, debug=False,
                   num_devices=NCORES)
    xT = nc.dram_tensor("xT", [E, R], F32R, kind="ExternalInput")
    wTk = nc.dram_tensor("wTk", [E, D], F32R, kind="ExternalInput")
    wTv = nc.dram_tensor("wTv", [E, D], F32R, kind="ExternalInput")
    wTkl = nc.dram_tensor("wTkl", [E, D], F32R, kind="ExternalInput")
    wTvl = nc.dram_tensor("wTvl", [E, D], F32R, kind="ExternalInput")
    bk = nc.dram_tensor("bk", [D, 1], F32, kind="ExternalInput")
    bkl = nc.dram_tensor("bkl", [D, 1], F32, kind="ExternalInput")
    bv16 = nc.dram_tensor("bv16", [1, D], F16, kind="ExternalInput")
    bvl16 = nc.dram_tensor("bvl16", [1, D], F16, kind="ExternalInput")
    ones16 = nc.dram_tensor("ones16", [1, 128], F16, kind="ExternalInput")
    ident16 = nc.dram_tensor("ident16", [128, 128], F16, kind="ExternalInput")
    kfm = nc.dram_tensor("kfm", [D, R], F16, kind="ExternalOutput")
    klfm = nc.dram_tensor("klfm", [D, R], F16, kind="ExternalOutput")
    vtm = nc.dram_tensor("vtm", [R, D], F16, kind="ExternalOutput")
    vltm = nc.dram_tensor("vltm", [R, D], F16, kind="ExternalOutput")
    # kv chunk states: per l, per batch a (128, 256) block grid, see col map
    kvch = nc.dram_tensor("kvch", [LSEG, 128, 1024], F32, kind="ExternalOutput")

    xT_t = xT[:].rearrange("(a p) r -> p a r", p=128)      # (128, 8, R)
    kfm_t = kfm[:].rearrange("(g p) r -> p g r", p=128)    # (128, 4, R)
    klfm_t = klfm[:].rearrange("(g p) r -> p g r", p=128)
    w_t = {k: v[:].rearrange("(a p) d -> p a d", p=128)
           for k, v in (("k", wTk), ("v", wTv), ("kl", wTkl), ("vl", wTvl))}
    bk_t = bk[:].rearrange("(g p) o -> p (g o)", p=128)    # (128, 4)
    bkl_t = bkl[:].rearrange("(g p) o -> p (g o)", p=128)

    with tile.TileContext(nc) as tc:
        with (
            tc.tile_pool(name="const", bufs=1) as cst,
            tc.tile_pool(name="xt", bufs=2) as pxt,
            tc.tile_pool(name="act", bufs=3) as pact,
            tc.tile_pool(name="ps", bufs=2, space="PSUM") as pps,
            tc.tile_pool(name="pst", bufs=2, space="PSUM") as ppt,
            tc.tile_pool(name="pskv", bufs=2, space="PSUM") as ppk,
        ):
            wk_sb = cst.tile([128, 8, D], F32R)
            wv_sb = cst.tile([128, 8, D], F32R)
            wkl_sb = cst.tile([128, 8, D], F32R)
            wvl_sb = cst.tile([128, 8, D], F32R)
            bk_sb = cst.tile([128, 4], F32)
            bkl_sb = cst.tile([128, 4], F32)
            bv_sb = cst.tile([1, D], F16)
            bvl_sb = cst.tile([1, D], F16)
            on_sb = cst.tile([1, 128], F16)
            id_sb = cst.tile([128, 128], F16)
            nc.sync.dma_start(out=wk_sb[:], in_=w_t["k"])
            nc.sync.dma_start(out=wv_sb[:], in_=w_t["v"])
            nc.sync.dma_start(out=wkl_sb[:], in_=w_t["kl"])
            nc.sync.dma_start(out=wvl_sb[:], in_=w_t["vl"])
            nc.sync.dma_start(out=bk_sb[:], in_=bk_t)
            nc.sync.dma_start(out=bkl_sb[:], in_=bkl_t)
            nc.sync.dma_start(out=bv_sb[:], in_=bv16[:])
            nc.sync.dma_start(out=bvl_sb[:], in_=bvl16[:])
            nc.sync.dma_start(out=on_sb[:], in_=ones16[:])
            nc.sync.dma_start(out=id_sb[:], in_=ident16[:])

            for rt in range(NRT):
                b, half = rt // 2, rt % 2
                r0 = b * SEG + half * RT

                xt = pxt.tile([128, 8, RT], F32R, tag="xt")
                nc.sync.dma_start(out=xt[:], in_=xT_t[:, :, r0:r0 + RT])

                # ---- feature-major k projections (k_lin gelu, k_loc plain) ----
                def proj_fm(w_sb, bias_sb, func, tag):
                    out16 = pact.tile([128, 4, RT], F16, tag=tag,
                                      name=f"{tag}_{rt}")
                    for gp in range(2):
                        ps = pps.tile([128, 512], F32, tag="ps",
                                      name=f"ps_{tag}_{rt}_{gp}")
                        for gh in range(2):
                            g = gp * 2 + gh
                            reg = ps[:, gh * RT:(gh + 1) * RT]
                            for et in range(8):
                                _mm(nc, reg, w_sb[:, et, g * 128:(g + 1) * 128],
                                    xt[:, et, :], start=(et == 0), stop=(et == 7))
                            nc.scalar.activation(out16[:, g, :], reg, func,
                                                 bias=bias_sb[:, g:g + 1],
                                                 scale=1.0)
                    return out16

                kf16 = proj_fm(wk_sb, bk_sb, AF.Gelu_apprx_tanh, "kf")
                kl16 = proj_fm(wkl_sb, bkl_sb, AF.Identity, "kl")
                nc.sync.dma_start(out=kfm_t[:, :, r0:r0 + RT], in_=kf16[:])
                nc.sync.dma_start(out=klfm_t[:, :, r0:r0 + RT], in_=kl16[:])

                # ---- transpose k_lin -> token-major (for kv only) ----
                kt16 = pact.tile([128, 2, D], F16, tag="kt")
                for g in range(4):
                    for rs in range(2):
                        pt = ppt.tile([128, 128], F16, tag="pt",
                                      name=f"pt_{rt}_{g}_{rs}")
                        nc.tensor.transpose(
                            pt[:], kf16[:, g, rs * 128:(rs + 1) * 128], id_sb[:])
                        nc.vector.tensor_copy(
                            kt16[:, rs, g * 128:(g + 1) * 128], pt[:])

                # ---- token-major v projections (bias via K=1 rank-1) ----
                def proj_tm(w_sb, bias_sb, dram, tag):
                    vt16 = pact.tile([128, 2, D], F16, tag=tag,
                                     name=f"{tag}_{rt}")
                    for rs in range(2):
                        ps = pps.tile([128, 512], F32, tag="ps",
                                      name=f"ps_{tag}_{rt}_{rs}")
                        for et in range(8):
                            _mm(nc, ps[:], xt[:, et, rs * 128:(rs + 1) * 128],
                                w_sb[:, et, :], start=(et == 0), stop=False)
                        _mm(nc, ps[:], on_sb[0:1, 0:128], bias_sb[:],
                            start=False, stop=True, skip=True)
                        nc.scalar.activation(vt16[:, rs, :], ps[:], AF.Copy)
                        nc.sync.dma_start(
                            out=dram[r0 + rs * 128:r0 + rs * 128 + 128, :],
                            in_=vt16[:, rs, :])
                    return vt16

                vt16 = proj_tm(wv_sb, bv_sb, vtm, "vt")
                proj_tm(wvl_sb, bvl_sb, vltm, "vl")

                # ---- per-chunk kv outer products (K=64) ----
                # col strips are parity-disjoint across row parities:
                # col = (2*(h%2) + l%2)*32, free offset = (h//2)*32.
                for cc in range(4):
                    l = 4 * half + cc
                    lp = cc % 2
                    pk = ppk.tile([128, 256], F32, tag="pk",
                                  name=f"pk_{rt}_{cc}")
                    for h in range(H):
                        colp = (2 * (h % 2) + lp) * 32
                        _mm(nc, pk[colp:colp + 32,
                                   (h // 2) * 32:(h // 2) * 32 + 32],
                            kt16[lp * 64:lp * 64 + 64, cc // 2,
                                 h * 32:h * 32 + 32],
                            vt16[lp * 64:lp * 64 + 64, cc // 2,
                                 h * 32:h * 32 + 32],
                            start=True, stop=True,
                            tp=(lp * 64, colp))
                    kv32 = pact.tile([128, 256], F32, tag="kv",
                                     name=f"kv_{rt}_{cc}")
                    nc.vector.tensor_copy(kv32[:], pk[:])
                    nc.sync.dma_start(out=kvch[l, :, b * 256:(b + 1) * 256],
                                      in_=kv32[:])
    nc.compile()
    return nc


def _build_launch2():
    nc = bacc.Bacc("TRN2", target_bir_lowering=False, debug=False,
                   num_devices=NCORES)
    xT = nc.dram_tensor("xT", [E, R], F32R, kind="ExternalInput")
    kpadA = nc.dram_tensor("kpadA", [128, H, R], F16, kind="ExternalInput")
    kpadB = nc.dram_tensor("kpadB", [128, H, R], F16, kind="ExternalInput")
    vpadA = nc.dram_tensor("vpadA", [128, B * LSEG, 512], F16, kind="ExternalInput")
    vpadB = nc.dram_tensor("vpadB", [128, B * LSEG, 512], F16, kind="ExternalInput")
    kvppad = nc.dram_tensor("kvppad", [128, LSEG, 2048], F16, kind="ExternalInput")
    wTq = nc.dram_tensor("wTq", [E, D], F32R, kind="ExternalInput")
    wTql = nc.dram_tensor("wTql", [E, D], F32R, kind="ExternalInput")
    bq = nc.dram_tensor("bq", [D, 1], F32, kind="ExternalInput")
    bql = nc.dram_tensor("bql", [D, 1], F32, kind="ExternalInput")
    wo1 = nc.dram_tensor("wo1", [D, E], F16, kind="ExternalInput")
    wo2 = nc.dram_tensor("wo2", [D, E], F16, kind="ExternalInput")
    ob16 = nc.dram_tensor("ob16", [1, E], F16, kind="ExternalInput")
    gate = nc.dram_tensor("gate", [D, 1], F32, kind="ExternalInput")
    mask = nc.dram_tensor("mask", [128, 512], F32, kind="ExternalInput")
    ones16 = nc.dram_tensor("ones16", [1, 256], F16, kind="ExternalInput")
    onc16 = nc.dram_tensor("onc16", [128, 1], F16, kind="ExternalInput")
    onc32 = nc.dram_tensor("onc32", [128, 1], F32R, kind="ExternalInput")
    outT = nc.dram_tensor("outT", [E, R], F32, kind="ExternalOutput")

    xT_t = xT[:].rearrange("(a p) r -> p a r", p=128)
    wq_t = wTq[:].rearrange("(a p) d -> p a d", p=128)
    wql_t = wTql[:].rearrange("(a p) d -> p a d", p=128)
    wo1_t = wo1[:].rearrange("(g p) e -> p g e", p=128)
    wo2_t = wo2[:].rearrange("(g p) e -> p g e", p=128)
    bq_t = bq[:].rearrange("(g p) o -> p (g o)", p=128)
    bql_t = bql[:].rearrange("(g p) o -> p (g o)", p=128)
    gate_t = gate[:].rearrange("(g p) o -> p (g o)", p=128)

    with tile.TileContext(nc) as tc:
        with (
            tc.tile_pool(name="const", bufs=1) as cst,
            tc.tile_pool(name="xt", bufs=2) as pxt,
            tc.tile_pool(name="act", bufs=2) as pact,
            tc.tile_pool(name="sc", bufs=2) as psc,
            tc.tile_pool(name="o16", bufs=2) as po,
            tc.tile_pool(name="rows", bufs=1) as prow,
            tc.tile_pool(name="psproj", bufs=2, space="PSUM") as ppj,
            tc.tile_pool(name="pssc", bufs=2, space="PSUM") as ppsc,
            tc.tile_pool(name="psst", bufs=1, space="PSUM") as ppst,
            tc.tile_pool(name="pso", bufs=1, space="PSUM") as ppo,
        ):
            wq_sb = cst.tile([128, 8, D], F32R)
            wql_sb = cst.tile([128, 8, D], F32R)
            kvp_sb = cst.tile([128, LSEG, 2048], F16)
            wo1_sb = cst.tile([128, 4, E], F16)
            wo2_sb = cst.tile([128, 4, E], F16)
            bq_sb = cst.tile([128, 4], F32)
            bql_sb = cst.tile([128, 4], F32)
            gate_sb = cst.tile([128, 4], F32)
            ob_sb = cst.tile([1, E], F16)
            on_sb = cst.tile([1, 256], F16)
            onc_sb = cst.tile([128, 1], F16)
            onc32_sb = cst.tile([128, 1], F32R)
            epsA = cst.tile([1, 1], F32)
            epsB = cst.tile([1, 1], F32)
            nc.vector.memset(epsA[:], 1e-5)
            nc.vector.memset(epsB[:], 1e-8)
            mk_sb = cst.tile([128, 512], F32)
            nc.sync.dma_start(out=wq_sb[:], in_=wq_t)
            nc.sync.dma_start(out=wql_sb[:], in_=wql_t)
            nc.sync.dma_start(out=kvp_sb[:], in_=kvppad[:])
            nc.sync.dma_start(out=wo1_sb[:], in_=wo1_t)
            nc.sync.dma_start(out=wo2_sb[:], in_=wo2_t)
            nc.sync.dma_start(out=bq_sb[:], in_=bq_t)
            nc.sync.dma_start(out=bql_sb[:], in_=bql_t)
            nc.sync.dma_start(out=gate_sb[:], in_=gate_t)
            nc.sync.dma_start(out=ob_sb[:], in_=ob16[:])
            nc.sync.dma_start(out=on_sb[:], in_=ones16[:])
            nc.sync.dma_start(out=onc_sb[:], in_=onc16[:])
            nc.sync.dma_start(out=onc32_sb[:], in_=onc32[:])
            nc.sync.dma_start(out=mk_sb[:], in_=mask[:])

            for rt in range(NRT):
                b, half = rt // 2, rt % 2
                r0 = b * SEG + half * RT
                xt = pxt.tile([128, 8, RT], F32R, tag="xt")
                nc.sync.dma_start(out=xt[:], in_=xT_t[:, :, r0:r0 + RT])
                kpA = pact.tile([128, H, RT], F16, tag="kpA")
                nc.sync.dma_start(out=kpA[:], in_=kpadA[:, :, r0:r0 + RT])
                kpB = pact.tile([128, H, RT], F16, tag="kpB")
                nc.sync.dma_start(out=kpB[:], in_=kpadB[:, :, r0:r0 + RT])
                bl0 = b * LSEG + 4 * half
                vpA = pact.tile([128, 4, 512], F16, tag="vpA")
                nc.sync.dma_start(out=vpA[:], in_=vpadA[:, bl0:bl0 + 4, :])
                vpB = pact.tile([128, 4, 512], F16, tag="vpB")
                nc.sync.dma_start(out=vpB[:], in_=vpadB[:, bl0:bl0 + 4, :])

                # ---- q projections ----
                def proj_fm(w_sb, bias_sb, func, tag):
                    out16 = pact.tile([128, 4, RT], F16, tag=tag,
                                      name=f"{tag}_{rt}")
                    for gp in range(2):
                        ps = ppj.tile([128, 512], F32, tag="pj",
                                      name=f"pj_{tag}_{rt}_{gp}")
                        for gh in range(2):
                            g = gp * 2 + gh
                            reg = ps[:, gh * RT:(gh + 1) * RT]
                            for et in range(8):
                                _mm(nc, reg,
                                    w_sb[:, et, g * 128:(g + 1) * 128],
                                    xt[:, et, :],
                                    start=(et == 0), stop=(et == 7))
                            nc.scalar.activation(out16[:, g, :], reg, func,
                                                 bias=bias_sb[:, g:g + 1],
                                                 scale=1.0)
                    return out16

                qf16 = proj_fm(wq_sb, bq_sb, AF.Gelu_apprx_tanh, "qf")
                ql16 = proj_fm(wql_sb, bql_sb, AF.Identity, "ql")

                # ---- attention ----
                xh1 = po.tile([128, 2, 512], F16, tag="xh1")
                xh2 = po.tile([128, 2, 512], F16, tag="xh2")
                o1_16 = po.tile([128, 2, 512], F16, tag="o1")
                o2_16 = po.tile([128, 2, 512], F16, tag="o2")

                for hp in range(2):
                    po1 = ppo.tile([128, 512], F32, tag="po1",
                                   name=f"po1_{rt}_{hp}")
                    po2 = ppo.tile([128, 512], F32, tag="po2",
                                   name=f"po2_{rt}_{hp}")
                    for hgh in range(2):
                        hg = hp * 2 + hgh
                        scm = psc.tile([128, 512], F16, tag="scm",
                                       name=f"scm_{rt}_{hp}_{hgh}")
                        scml = psc.tile([128, 512], F16, tag="scml",
                                        name=f"scml_{rt}_{hp}_{hgh}")
                        for br, (kk, qq, sm) in enumerate(
                            ((kpA, qf16, scm), (kpB, ql16, scml))
                        ):
                            psx = ppsc.tile([128, 512], F32, tag="psx",
                                            name=f"psx_{rt}_{hp}_{hgh}_{br}")
                            for cc in range(4):
                                jj, lp = cc // 2, cc % 2
                                for s in range(4):
                                    h = hg * 4 + s
                                    for mh in range(2):
                                        _mm(nc, psx[lp * 64 + mh * 32:
                                                    lp * 64 + mh * 32 + 32,
                                                    jj * 256 + s * 64:
                                                    jj * 256 + s * 64 + 64],
                                            kk[:, h,
                                               cc * 64 + mh * 32:
                                               cc * 64 + mh * 32 + 32],
                                            qq[:, hg, cc * 64:cc * 64 + 64],
                                            start=True, stop=True,
                                            tp=(0, lp * 64 + mh * 32))
                            op0 = ALU.bypass if br == 0 else ALU.max
                            nc.vector.scalar_tensor_tensor(
                                sm[:], psx[:], 0.0, mk_sb[:],
                                op0=op0, op1=ALU.mult)

                        # second matmuls (all K=128 via zero-padded lhsT)
                        for cc in range(4):
                            jj, lp = cc // 2, cc % 2
                            l = 4 * half + cc
                            for s in range(4):
                                h = hg * 4 + s
                                co = hgh * 256 + cc * 64
                                _mm(nc, po1[s * 32:s * 32 + 32, co:co + 64],
                                    vpA[:, cc, h * 32:h * 32 + 32],
                                    scm[:, jj * 256 + s * 64:jj * 256 + s * 64 + 64],
                                    start=True, stop=False, tp=(0, s * 32))
                                _mm(nc, po1[s * 32:s * 32 + 32, co:co + 64],
                                    kvp_sb[:, l, b * 512 + h * 32:b * 512 + h * 32 + 32],
                                    qf16[:, hg, cc * 64:cc * 64 + 64],
                                    start=False, stop=True, tp=(0, s * 32))
                                _mm(nc, po2[s * 32:s * 32 + 32, co:co + 64],
                                    vpB[:, cc, h * 32:h * 32 + 32],
                                    scml[:, jj * 256 + s * 64:jj * 256 + s * 64 + 64],
                                    start=True, stop=True, tp=(0, s * 32))
                    nc.vector.tensor_copy(o1_16[:, hp, :], po1[:])
                    nc.vector.tensor_copy(o2_16[:, hp, :], po2[:])

                # ---- stats (column sums via ones-column matmuls) ----
                sq1 = [psc.tile([128, 512], F32R, tag="sq1", name=f"sq1_{rt}_{i}") for i in range(2)]
                sq2 = [psc.tile([128, 512], F32R, tag="sq2", name=f"sq2_{rt}_{i}") for i in range(2)]
                for hp in range(2):
                    nc.scalar.activation(sq1[hp][:], o1_16[:, hp, :], AF.Square)
                    nc.scalar.activation(sq2[hp][:], o2_16[:, hp, :], AF.Square)
                stA = ppst.tile([1, 512], F32, tag="stA", name=f"stA_{rt}")
                stB = ppst.tile([1, 512], F32, tag="stB", name=f"stB_{rt}")
                for g in range(4):
                    hp, gh = g // 2, g % 2
                    sl = slice(gh * 256, gh * 256 + 256)
                    _mm(nc, stA[0:1, 0:256], onc_sb[:],
                        o1_16[:, hp, sl], start=(g == 0), stop=(g == 3), skip=True)
                for g in range(4):
                    hp, gh = g // 2, g % 2
                    sl = slice(gh * 256, gh * 256 + 256)
                    _mm(nc, stA[0:1, 256:512], onc32_sb[:],
                        sq1[hp][:, sl], start=(g == 0), stop=(g == 3), skip=True)
                for g in range(4):
                    hp, gh = g // 2, g % 2
                    sl = slice(gh * 256, gh * 256 + 256)
                    _mm(nc, stB[0:1, 0:256], onc32_sb[:],
                        sq2[hp][:, sl], start=(g == 0), stop=(g == 3), skip=True)

                rows = prow.tile([1, 1792], F32, tag="rows", name=f"rows_{rt}")
                mu = rows[0:1, 0:256]
                msq = rows[0:1, 256:512]
                var = rows[0:1, 512:768]
                rstd = rows[0:1, 768:1024]
                inv2 = rows[0:1, 1024:1280]
                nc.scalar.activation(mu, stA[0:1, 0:256], AF.Copy, scale=1.0 / D)
                nc.scalar.activation(msq, stA[0:1, 256:512], AF.Copy, scale=1.0 / D)
                nc.vector.scalar_tensor_tensor(var, mu, 0.0, mu,
                                               op0=ALU.bypass, op1=ALU.mult)
                nc.vector.tensor_tensor(var, msq, var, op=ALU.subtract)
                sq_a = rows[0:1, 1280:1536]
                sq_b = rows[0:1, 1536:1792]
                nc.scalar.activation(sq_a, var, AF.Sqrt, bias=epsA[0:1, 0:1], scale=1.0)
                nc.vector.reciprocal(rstd, sq_a)
                nc.scalar.activation(sq_b, stB[0:1, 0:256], AF.Sqrt,
                                     bias=epsB[0:1, 0:1], scale=1.0 / D)
                nc.vector.reciprocal(inv2, sq_b)

                # ---- normalize (broadcast rows via K=1 ones x row matmuls) ----
                rows16 = prow.tile([1, 768], F16, tag="rows16", name=f"rows16_{rt}")
                nc.scalar.activation(rows16[0:1, 0:256], mu, AF.Copy)
                nc.scalar.activation(rows16[0:1, 256:512], rstd, AF.Copy)
                nc.scalar.activation(rows16[0:1, 512:768], inv2, AF.Copy)
                bc1 = ppsc.tile([128, 512], F32, tag="psx", name=f"bc1_{rt}")
                _mm(nc, bc1[:, 0:256], on_sb[0:1, 0:128],
                    rows16[0:1, 0:256], start=True, stop=True)
                _mm(nc, bc1[:, 256:512], on_sb[0:1, 0:128],
                    rows16[0:1, 256:512], start=True, stop=True)
                bc2 = ppsc.tile([128, 512], F32, tag="psx", name=f"bc2_{rt}")
                _mm(nc, bc2[:, 0:256], on_sb[0:1, 0:128],
                    rows16[0:1, 512:768], start=True, stop=True)
                for g in range(4):
                    hp, gh = g // 2, g % 2
                    sl = slice(gh * 256, gh * 256 + 256)
                    t16 = psc.tile([128, 256], F16, tag="t16",
                                   name=f"t16_{rt}_{g}")
                    nc.vector.tensor_tensor(t16[:], o1_16[:, hp, sl],
                                            bc1[:, 0:256], op=ALU.subtract)
                    nc.vector.tensor_tensor(xh1[:, hp, sl], t16[:],
                                            bc1[:, 256:512], op=ALU.mult)
                    sg16 = psc.tile([128, 256], F16, tag="sg16",
                                    name=f"sg16_{rt}_{g}")
                    nc.scalar.activation(sg16[:], o2_16[:, hp, sl], AF.Sigmoid,
                                         scale=gate_sb[:, g:g + 1])
                    u16 = psc.tile([128, 256], F16, tag="u16",
                                   name=f"u16_{rt}_{g}")
                    nc.vector.tensor_tensor(u16[:], o2_16[:, hp, sl], sg16[:],
                                            op=ALU.mult)
                    nc.vector.tensor_tensor(xh2[:, hp, sl], u16[:],
                                            bc2[:, 0:256], op=ALU.mult)

                # ---- out projections ----
                for ep in range(4):
                    pu = ppj.tile([128, 512], F32, tag="pj", name=f"pu_{rt}_{ep}")
                    for eh in range(2):
                        et = ep * 2 + eh
                        reg = pu[:, eh * 256:eh * 256 + 256]
                        for g in range(4):
                            hp, gh = g // 2, g % 2
                            sl = slice(gh * 256, gh * 256 + 256)
                            _mm(nc, reg, wo1_sb[:, g, et * 128:et * 128 + 128],
                                xh1[:, hp, sl], start=(g == 0), stop=False)
                        for g in range(4):
                            hp, gh = g // 2, g % 2
                            sl = slice(gh * 256, gh * 256 + 256)
                            _mm(nc, reg, wo2_sb[:, g, et * 128:et * 128 + 128],
                                xh2[:, hp, sl], start=False, stop=False)
                        _mm(nc, reg, ob_sb[0:1, et * 128:et * 128 + 128],
                            on_sb[:], start=False, stop=True, skip=True)
                    ot32 = po.tile([128, 512], F32, tag="ot", name=f"ot_{rt}_{ep}")
                    nc.vector.tensor_copy(ot32[:], pu[:])
                    for eh in range(2):
                        et = ep * 2 + eh
                        nc.sync.dma_start(
                            out=outT[et * 128:et * 128 + 128, r0:r0 + RT],
                            in_=ot32[:, eh * 256:eh * 256 + 256])
    nc.compile()
    return nc


def _get(name):
    if name not in _CACHE:
        _CACHE[name] = _build_launch1() if name == "l1" else _build_launch2()
    return _CACHE[name]


def kernel(**inputs):
    inp = {k: np.asarray(v, np.float32) for k, v in inputs.items()}
    q = inp["query"]                                     # (N, B, E)
    xT_all = np.ascontiguousarray(q.transpose(2, 1, 0))  # (E, B, N)
    xT_cores = [
        np.ascontiguousarray(xT_all[:, :, s * SEG:(s + 1) * SEG].reshape(E, R))
        for s in range(NCORES)
    ]
    wT = {f"{p}_{t}": np.ascontiguousarray(inp[f"w{p}_{t}"].T)
          for t in ("lin", "loc") for p in ("q", "k", "v")}
    g, bln = inp["ln_g"], inp["ln_b"]
    wo1 = 0.5 * (inp["wo_lin"] * g[None, :])
    bias1 = 0.5 * (inp["wo_lin"] @ bln + inp["bo_lin"])
    wo2 = 0.5 * (inp["wo_loc"] * inp["grn_scale"][None, :])
    bias2 = 0.5 * inp["bo_loc"]
    obias = (bias1 + bias2).astype(np.float16)[None, :]

    ones128 = np.ones((1, 128), np.float16)
    ones256 = np.ones((1, 256), np.float16)
    ident = np.eye(128, dtype=np.float16)
    mask = np.tile(np.triu(np.ones((C, C), np.float32)), (2, 8))

    # ---- launch 1: k/v projections for both branches + chunk kv states ----
    nc1 = _get("l1")
    in1 = [{
        "xT": xT_cores[s],
        "wTk": wT["k_lin"], "wTv": wT["v_lin"],
        "wTkl": wT["k_loc"], "wTvl": wT["v_loc"],
        "bk": inp["bk_lin"][:, None].astype(np.float32),
        "bkl": inp["bk_loc"][:, None].astype(np.float32),
        "bv16": inp["bv_lin"][None, :].astype(np.float16),
        "bvl16": inp["bv_loc"][None, :].astype(np.float16),
        "ones16": ones128, "ident16": ident,
    } for s in range(NCORES)]
    LAST_EXEC_NS.clear()
    r1 = run_bass_kernel_spmd(nc1, in1, list(range(NCORES)), trace=TRACE)
    res1 = r1.results
    if TRACE:
        LAST_EXEC_NS.append(r1.exec_time_ns)

    # ---- host: decode kv blocks, exclusive prefix over 64 global chunks ----
    # kvch[l, (2*(h%2)+(l%2))*32 + d, b*256 + (h//2)*32 + e] = kv[b,h,g,d,e]
    kv_all = np.zeros((NCORES * LSEG, B, H, 32, 32), np.float32)
    for s in range(NCORES):
        kvch = res1[s]["kvch"]                  # (LSEG, 128, 1024)
        for l in range(LSEG):
            for h in range(H):
                colp = (2 * (h % 2) + (l % 2)) * 32
                blk = kvch[l, colp:colp + 32].reshape(32, 4, 8, 32)
                kv_all[s * LSEG + l, :, h] = blk[:, :, h // 2, :].transpose(1, 0, 2)
    cum = np.cumsum(kv_all, axis=0)
    kvp_all = np.concatenate(
        [np.zeros((1, B, H, 32, 32), np.float32), cum[:-1]], axis=0)

    # padded per-core containers for launch 2
    def make_kpad(kfm):
        kp = np.zeros((128, H, R), np.float16)
        for h in range(H):
            kp[(h % 4) * 32:(h % 4) * 32 + 32, h, :] = kfm[h * 32:(h + 1) * 32, :]
        return kp

    def make_vpad(vtm):
        vp = np.zeros((128, B * LSEG, 512), np.float16)
        for b2 in range(B):
            for l in range(LSEG):
                vp[(l % 2) * 64:(l % 2) * 64 + 64, b2 * LSEG + l, :] = (
                    vtm[b2 * SEG + l * C:b2 * SEG + l * C + C, :])
        return vp

    def make_kvppad(kvp_seg):
        kp = np.zeros((128, LSEG, 2048), np.float16)
        for b2 in range(B):
            for h in range(H):
                kp[(h % 4) * 32:(h % 4) * 32 + 32, :,
                   b2 * 512 + h * 32:b2 * 512 + h * 32 + 32] = (
                    kvp_seg[:, b2, h].transpose(1, 0, 2))
        return kp

    # ---- launch 2 ----
    nc2 = _get("l2")
    in2 = [{
        "xT": xT_cores[s],
        "kpadA": make_kpad(res1[s]["kfm"]),
        "kpadB": make_kpad(res1[s]["klfm"]),
        "vpadA": make_vpad(res1[s]["vtm"]),
        "vpadB": make_vpad(res1[s]["vltm"]),
        "kvppad": make_kvppad(kvp_all[s * LSEG:(s + 1) * LSEG]),
        "wTq": wT["q_lin"], "wTql": wT["q_loc"],
        "bq": inp["bq_lin"][:, None].astype(np.float32),
        "bql": inp["bq_loc"][:, None].astype(np.float32),
        "wo1": np.ascontiguousarray(wo1.T).astype(np.float16),
        "wo2": np.ascontiguousarray(wo2.T).astype(np.float16),
        "ob16": obias,
        "gate": inp["grn_gate"][:, None].astype(np.float32),
        "mask": mask, "ones16": ones256,
        "onc16": np.ones((128, 1), np.float16),
        "onc32": np.ones((128, 1), np.float32),
    } for s in range(NCORES)]
    r2 = run_bass_kernel_spmd(nc2, in2, list(range(NCORES)), trace=TRACE)
    res2 = r2.results
    if TRACE:
        LAST_EXEC_NS.append(r2.exec_time_ns)

    full = np.zeros((N, B, E), np.float32)
    for s in range(NCORES):
        o = res2[s]["outT"].reshape(E, B, SEG)
        full[s * SEG:(s + 1) * SEG] = o.transpose(2, 1, 0)
    return full


# revision 21
# speedup vs baseline: 1.0366x; 1.0366x over previous
"""NormMixAttention Trainium2 kernel — 8-core SPMD, sequence-sharded.

Strategy: shard the 4096-token sequence into 8 segments of 512 tokens (all 4
batches on every core; 2048 token-columns per core). Activations live
feature-major on chip ([d on partitions, tokens on free]); the host
pre-transposes query and the weights so no on-chip input transpose is needed.

Two SPMD launches (the linear branch's inter-chunk kv recurrence crosses
segment boundaries):
  launch 1: k_lin (gelu) + v_lin projections, per-chunk kv outer products.
  host:     exclusive prefix-sum of the 64 per-chunk kv states (tiny).
  launch 2: everything else (q_lin, local branch, intra+inter attention,
            layernorm / gated rmsnorm, fused out-projections).

Matmul dtypes: projections in float32r (full PE rate, tf32-like precision),
attention-block and out-proj matmuls in fp16 (small free dims), PSUM fp32.
LN gamma/beta, grn scale and the 0.5 branch-average are folded into the
out-projection weights/bias on the host (exact algebra).
"""
import numpy as np

import concourse.bass as bass
import concourse.mybir as mybir
import concourse.tile as tile
from concourse import bacc
from concourse.bass_utils import run_bass_kernel_spmd

F32, F32R, F16 = mybir.dt.float32, mybir.dt.float32r, mybir.dt.float16
AF = mybir.ActivationFunctionType
from concourse.alu_op_type import AluOpType as ALU

N, B, E, H, C = 4096, 4, 1024, 16, 64
D = E // 2            # 512
DH = D // H           # 32
NCORES = 8
SEG = N // NCORES     # 512 tokens/core
LSEG = SEG // C       # 8 chunks/core/batch
R = B * SEG           # 2048 columns/core
RT = 256              # r-tile width
NRT = R // RT         # 8 r-tiles/core

_CACHE = {}
TRACE = False
LAST_EXEC_NS = []


def _mm(nc, out, lhsT, rhs, start, stop, tp=None, skip=False):
    if tp is not None and tp == (0, 0):
        tp = None
    nc.tensor.matmul(out, lhsT, rhs, start=start, stop=stop, tile_position=tp,
                     skip_group_check=skip)


def _build_launch1():
    nc = bacc.Bacc("TRN2", target_bir_lowering=False, debug=False,
                   num_devices=NCORES)
    xT = nc.dram_tensor("xT", [E, R], F32R, kind="ExternalInput")
    wTk = nc.dram_tensor("wTk", [E, D], F32R, kind="ExternalInput")
    wTv = nc.dram_tensor("wTv", [E, D], F32R, kind="ExternalInput")
    wTkl = nc.dram_tensor("wTkl", [E, D], F32R, kind="ExternalInput")
    wTvl = nc.dram_tensor("wTvl", [E, D], F32R, kind="ExternalInput")
    bk = nc.dram_tensor("bk", [D, 1], F32, kind="ExternalInput")
    bkl = nc.dram_tensor("bkl", [D, 1], F32, kind="ExternalInput")
    bv16 = nc.dram_tensor("bv16", [1, D], F16, kind="ExternalInput")
    bvl16 = nc.dram_tensor("bvl16", [1, D], F16, kind="ExternalInput")
    ones16 = nc.dram_tensor("ones16", [1, 128], F16, kind="ExternalInput")
    ident16 = nc.dram_tensor("ident16", [128, 128], F16, kind="ExternalInput")
    kfm = nc.dram_tensor("kfm", [D, R], F16, kind="ExternalOutput")
    klfm = nc.dram_tensor("klfm", [D, R], F16, kind="ExternalOutput")
    vtm = nc.dram_tensor("vtm", [R, D], F16, kind="ExternalOutput")
    vltm = nc.dram_tensor("vltm", [R, D], F16, kind="ExternalOutput")
    # kv chunk states: per l, per batch a (128, 256) block grid, see col map
    kvch = nc.dram_tensor("kvch", [LSEG, 128, 1024], F32, kind="ExternalOutput")

    xT_t = xT[:].rearrange("(a p) r -> p a r", p=128)      # (128, 8, R)
    kfm_t = kfm[:].rearrange("(g p) r -> p g r", p=128)    # (128, 4, R)
    klfm_t = klfm[:].rearrange("(g p) r -> p g r", p=128)
    w_t = {k: v[:].rearrange("(a p) d -> p a d", p=128)
           for k, v in (("k", wTk), ("v", wTv), ("kl", wTkl), ("vl", wTvl))}
    bk_t = bk[:].rearrange("(g p) o -> p (g o)", p=128)    # (128, 4)
    bkl_t = bkl[:].rearrange("(g p) o -> p (g o)", p=128)

    with tile.TileContext(nc) as tc:
        with (
            tc.tile_pool(name="const", bufs=1) as cst,
            tc.tile_pool(name="xt", bufs=2) as pxt,
            tc.tile_pool(name="act", bufs=3) as pact,
            tc.tile_pool(name="ps", bufs=2, space="PSUM") as pps,
            tc.tile_pool(name="pst", bufs=2, space="PSUM") as ppt,
            tc.tile_pool(name="pskv", bufs=2, space="PSUM") as ppk,
        ):
            wk_sb = cst.tile([128, 8, D], F32R)
            wv_sb = cst.tile([128, 8, D], F32R)
            wkl_sb = cst.tile([128, 8, D], F32R)
            wvl_sb = cst.tile([128, 8, D], F32R)
            bk_sb = cst.tile([128, 4], F32)
            bkl_sb = cst.tile([128, 4], F32)
            bv_sb = cst.tile([1, D], F16)
            bvl_sb = cst.tile([1, D], F16)
            on_sb = cst.tile([1, 128], F16)
            id_sb = cst.tile([128, 128], F16)
            nc.sync.dma_start(out=wk_sb[:], in_=w_t["k"])
            nc.sync.dma_start(out=wv_sb[:], in_=w_t["v"])
            nc.sync.dma_start(out=wkl_sb[:], in_=w_t["kl"])
            nc.sync.dma_start(out=wvl_sb[:], in_=w_t["vl"])
            nc.sync.dma_start(out=bk_sb[:], in_=bk_t)
            nc.sync.dma_start(out=bkl_sb[:], in_=bkl_t)
            nc.sync.dma_start(out=bv_sb[:], in_=bv16[:])
            nc.sync.dma_start(out=bvl_sb[:], in_=bvl16[:])
            nc.sync.dma_start(out=on_sb[:], in_=ones16[:])
            nc.sync.dma_start(out=id_sb[:], in_=ident16[:])

            for rt in range(NRT):
                b, half = rt // 2, rt % 2
                r0 = b * SEG + half * RT

                xt = pxt.tile([128, 8, RT], F32R, tag="xt")
                nc.sync.dma_start(out=xt[:], in_=xT_t[:, :, r0:r0 + RT])

                # ---- feature-major k projections (k_lin gelu, k_loc plain) ----
                def proj_fm(w_sb, bias_sb, func, tag):
                    out16 = pact.tile([128, 4, RT], F16, tag=tag,
                                      name=f"{tag}_{rt}")
                    for gp in range(2):
                        ps = pps.tile([128, 512], F32, tag="ps",
                                      name=f"ps_{tag}_{rt}_{gp}")
                        for gh in range(2):
                            g = gp * 2 + gh
                            reg = ps[:, gh * RT:(gh + 1) * RT]
                            for et in range(8):
                                _mm(nc, reg, w_sb[:, et, g * 128:(g + 1) * 128],
                                    xt[:, et, :], start=(et == 0), stop=(et == 7))
                            nc.scalar.activation(out16[:, g, :], reg, func,
                                                 bias=bias_sb[:, g:g + 1],
                                                 scale=1.0)
                    return out16

                kf16 = proj_fm(wk_sb, bk_sb, AF.Gelu_apprx_tanh, "kf")
                kl16 = proj_fm(wkl_sb, bkl_sb, AF.Identity, "kl")
                nc.sync.dma_start(out=kfm_t[:, :, r0:r0 + RT], in_=kf16[:])
                nc.sync.dma_start(out=klfm_t[:, :, r0:r0 + RT], in_=kl16[:])

                # ---- transpose k_lin -> token-major (for kv only) ----
                kt16 = pact.tile([128, 2, D], F16, tag="kt")
                for g in range(4):
                    for rs in range(2):
                        pt = ppt.tile([128, 128], F16, tag="pt",
                                      name=f"pt_{rt}_{g}_{rs}")
                        nc.tensor.transpose(
                            pt[:], kf16[:, g, rs * 128:(rs + 1) * 128], id_sb[:])
                        nc.vector.tensor_copy(
                            kt16[:, rs, g * 128:(g + 1) * 128], pt[:])

                # ---- token-major v projections (bias via K=1 rank-1) ----
                def proj_tm(w_sb, bias_sb, dram, tag):
                    vt16 = pact.tile([128, 2, D], F16, tag=tag,
                                     name=f"{tag}_{rt}")
                    for rs in range(2):
                        ps = pps.tile([128, 512], F32, tag="ps",
                                      name=f"ps_{tag}_{rt}_{rs}")
                        for et in range(8):
                            _mm(nc, ps[:], xt[:, et, rs * 128:(rs + 1) * 128],
                                w_sb[:, et, :], start=(et == 0), stop=False)
                        _mm(nc, ps[:], on_sb[0:1, 0:128], bias_sb[:],
                            start=False, stop=True, skip=True)
                        nc.scalar.activation(vt16[:, rs, :], ps[:], AF.Copy)
                        nc.sync.dma_start(
                            out=dram[r0 + rs * 128:r0 + rs * 128 + 128, :],
                            in_=vt16[:, rs, :])
                    return vt16

                vt16 = proj_tm(wv_sb, bv_sb, vtm, "vt")
                proj_tm(wvl_sb, bvl_sb, vltm, "vl")

                # ---- per-chunk kv outer products (K=64) ----
                # col strips are parity-disjoint across row parities:
                # col = (2*(h%2) + l%2)*32, free offset = (h//2)*32.
                for cc in range(4):
                    l = 4 * half + cc
                    lp = cc % 2
                    pk = ppk.tile([128, 256], F32, tag="pk",
                                  name=f"pk_{rt}_{cc}")
                    for h in range(H):
                        colp = (2 * (h % 2) + lp) * 32
                        _mm(nc, pk[colp:colp + 32,
                                   (h // 2) * 32:(h // 2) * 32 + 32],
                            kt16[lp * 64:lp * 64 + 64, cc // 2,
                                 h * 32:h * 32 + 32],
                            vt16[lp * 64:lp * 64 + 64, cc // 2,
                                 h * 32:h * 32 + 32],
                            start=True, stop=True,
                            tp=(lp * 64, colp))
                    kv32 = pact.tile([128, 256], F32, tag="kv",
                                     name=f"kv_{rt}_{cc}")
                    nc.vector.tensor_copy(kv32[:], pk[:])
                    nc.sync.dma_start(out=kvch[l, :, b * 256:(b + 1) * 256],
                                      in_=kv32[:])
    nc.compile()
    return nc


def _build_launch2():
    nc = bacc.Bacc("TRN2", target_bir_lowering=False, debug=False,
                   num_devices=NCORES)
    xT = nc.dram_tensor("xT", [E, R], F32R, kind="ExternalInput")
    kpadA = nc.dram_tensor("kpadA", [128, H, R], F16, kind="ExternalInput")
    kpadB = nc.dram_tensor("kpadB", [128, H, R], F16, kind="ExternalInput")
    vpadA = nc.dram_tensor("vpadA", [128, B * LSEG, 512], F16, kind="ExternalInput")
    vpadB = nc.dram_tensor("vpadB", [128, B * LSEG, 512], F16, kind="ExternalInput")
    kvppad = nc.dram_tensor("kvppad", [128, LSEG, 2048], F16, kind="ExternalInput")
    wTq = nc.dram_tensor("wTq", [E, D], F32R, kind="ExternalInput")
    wTql = nc.dram_tensor("wTql", [E, D], F32R, kind="ExternalInput")
    bq = nc.dram_tensor("bq", [D, 1], F32, kind="ExternalInput")
    bql = nc.dram_tensor("bql", [D, 1], F32, kind="ExternalInput")
    wo1 = nc.dram_tensor("wo1", [D, E], F16, kind="ExternalInput")
    wo2 = nc.dram_tensor("wo2", [D, E], F16, kind="ExternalInput")
    ob16 = nc.dram_tensor("ob16", [1, E], F16, kind="ExternalInput")
    gate = nc.dram_tensor("gate", [D, 1], F32, kind="ExternalInput")
    mask = nc.dram_tensor("mask", [128, 512], F32, kind="ExternalInput")
    ones16 = nc.dram_tensor("ones16", [1, 256], F16, kind="ExternalInput")
    onc16 = nc.dram_tensor("onc16", [128, 1], F16, kind="ExternalInput")
    onc32 = nc.dram_tensor("onc32", [128, 1], F32R, kind="ExternalInput")
    outT = nc.dram_tensor("outT", [E, R], F32, kind="ExternalOutput")

    xT_t = xT[:].rearrange("(a p) r -> p a r", p=128)
    wq_t = wTq[:].rearrange("(a p) d -> p a d", p=128)
    wql_t = wTql[:].rearrange("(a p) d -> p a d", p=128)
    wo1_t = wo1[:].rearrange("(g p) e -> p g e", p=128)
    wo2_t = wo2[:].rearrange("(g p) e -> p g e", p=128)
    bq_t = bq[:].rearrange("(g p) o -> p (g o)", p=128)
    bql_t = bql[:].rearrange("(g p) o -> p (g o)", p=128)
    gate_t = gate[:].rearrange("(g p) o -> p (g o)", p=128)

    with tile.TileContext(nc) as tc:
        with (
            tc.tile_pool(name="const", bufs=1) as cst,
            tc.tile_pool(name="xt", bufs=2) as pxt,
            tc.tile_pool(name="act", bufs=2) as pact,
            tc.tile_pool(name="sc", bufs=2) as psc,
            tc.tile_pool(name="o16", bufs=2) as po,
            tc.tile_pool(name="rows", bufs=1) as prow,
            tc.tile_pool(name="psproj", bufs=2, space="PSUM") as ppj,
            tc.tile_pool(name="pssc", bufs=2, space="PSUM") as ppsc,
            tc.tile_pool(name="psst", bufs=1, space="PSUM") as ppst,
            tc.tile_pool(name="pso", bufs=1, space="PSUM") as ppo,
        ):
            wq_sb = cst.tile([128, 8, D], F32R)
            wql_sb = cst.tile([128, 8, D], F32R)
            kvp_sb = cst.tile([128, LSEG, 2048], F16)
            wo1_sb = cst.tile([128, 4, E], F16)
            wo2_sb = cst.tile([128, 4, E], F16)
            bq_sb = cst.tile([128, 4], F32)
            bql_sb = cst.tile([128, 4], F32)
            gate_sb = cst.tile([128, 4], F32)
            ob_sb = cst.tile([1, E], F16)
            on_sb = cst.tile([1, 256], F16)
            onc_sb = cst.tile([128, 1], F16)
            onc32_sb = cst.tile([128, 1], F32R)
            epsA = cst.tile([1, 1], F32)
            epsB = cst.tile([1, 1], F32)
            nc.vector.memset(epsA[:], 1e-5)
            nc.vector.memset(epsB[:], 1e-8)
            mk_sb = cst.tile([128, 512], F32)
            nc.sync.dma_start(out=wq_sb[:], in_=wq_t)
            nc.sync.dma_start(out=wql_sb[:], in_=wql_t)
            nc.sync.dma_start(out=kvp_sb[:], in_=kvppad[:])
            nc.sync.dma_start(out=wo1_sb[:], in_=wo1_t)
            nc.sync.dma_start(out=wo2_sb[:], in_=wo2_t)
            nc.sync.dma_start(out=bq_sb[:], in_=bq_t)
            nc.sync.dma_start(out=bql_sb[:], in_=bql_t)
            nc.sync.dma_start(out=gate_sb[:], in_=gate_t)
            nc.sync.dma_start(out=ob_sb[:], in_=ob16[:])
            nc.sync.dma_start(out=on_sb[:], in_=ones16[:])
            nc.sync.dma_start(out=onc_sb[:], in_=onc16[:])
            nc.sync.dma_start(out=onc32_sb[:], in_=onc32[:])
            nc.sync.dma_start(out=mk_sb[:], in_=mask[:])

            for rt in range(NRT):
                b, half = rt // 2, rt % 2
                r0 = b * SEG + half * RT
                xt = pxt.tile([128, 8, RT], F32R, tag="xt")
                nc.sync.dma_start(out=xt[:], in_=xT_t[:, :, r0:r0 + RT])
                kpA = pact.tile([128, H, RT], F16, tag="kpA")
                nc.sync.dma_start(out=kpA[:], in_=kpadA[:, :, r0:r0 + RT])
                kpB = pact.tile([128, H, RT], F16, tag="kpB")
                nc.sync.dma_start(out=kpB[:], in_=kpadB[:, :, r0:r0 + RT])
                bl0 = b * LSEG + 4 * half
                vpA = pact.tile([128, 4, 512], F16, tag="vpA")
                nc.sync.dma_start(out=vpA[:], in_=vpadA[:, bl0:bl0 + 4, :])
                vpB = pact.tile([128, 4, 512], F16, tag="vpB")
                nc.sync.dma_start(out=vpB[:], in_=vpadB[:, bl0:bl0 + 4, :])

                # ---- q projections ----
                def proj_fm(w_sb, bias_sb, func, tag):
                    out16 = pact.tile([128, 4, RT], F16, tag=tag,
                                      name=f"{tag}_{rt}")
                    for gp in range(2):
                        ps = ppj.tile([128, 512], F32, tag="pj",
                                      name=f"pj_{tag}_{rt}_{gp}")
                        for gh in range(2):
                            g = gp * 2 + gh
                            reg = ps[:, gh * RT:(gh + 1) * RT]
                            for et in range(8):
                                _mm(nc, reg,
                                    w_sb[:, et, g * 128:(g + 1) * 128],
                                    xt[:, et, :],
                                    start=(et == 0), stop=(et == 7))
                            nc.scalar.activation(out16[:, g, :], reg, func,
                                                 bias=bias_sb[:, g:g + 1],
                                                 scale=1.0)
                    return out16

                qf16 = proj_fm(wq_sb, bq_sb, AF.Gelu_apprx_tanh, "qf")
                ql16 = proj_fm(wql_sb, bql_sb, AF.Identity, "ql")

                # ---- attention ----
                xh1 = po.tile([128, 2, 512], F16, tag="xh1")
                xh2 = po.tile([128, 2, 512], F16, tag="xh2")
                o1_16 = po.tile([128, 2, 512], F16, tag="o1")
                o2_16 = po.tile([128, 2, 512], F16, tag="o2")

                for hp in range(2):
                    po1 = ppo.tile([128, 512], F32, tag="po1",
                                   name=f"po1_{rt}_{hp}")
                    po2 = ppo.tile([128, 512], F32, tag="po2",
                                   name=f"po2_{rt}_{hp}")
                    for hgh in range(2):
                        hg = hp * 2 + hgh
                        scm = psc.tile([128, 512], F16, tag="scm",
                                       name=f"scm_{rt}_{hp}_{hgh}")
                        scml = psc.tile([128, 512], F16, tag="scml",
                                        name=f"scml_{rt}_{hp}_{hgh}")
                        for br, (kk, qq, sm) in enumerate(
                            ((kpA, qf16, scm), (kpB, ql16, scml))
                        ):
                            psx = ppsc.tile([128, 512], F32, tag="psx",
                                            name=f"psx_{rt}_{hp}_{hgh}_{br}")
                            for cc in range(4):
                                jj, lp = cc // 2, cc % 2
                                for s in range(4):
                                    h = hg * 4 + s
                                    _mm(nc, psx[lp * 64:lp * 64 + 64,
                                                jj * 256 + s * 64:
                                                jj * 256 + s * 64 + 64],
                                        kk[:, h, cc * 64:cc * 64 + 64],
                                        qq[:, hg, cc * 64:cc * 64 + 64],
                                        start=True, stop=True,
                                        tp=(0, lp * 64))
                            op0 = ALU.bypass if br == 0 else ALU.max
                            nc.vector.scalar_tensor_tensor(
                                sm[:], psx[:], 0.0, mk_sb[:],
                                op0=op0, op1=ALU.mult)

                        # second matmuls (all K=128 via zero-padded lhsT)
                        for cc in range(4):
                            jj, lp = cc // 2, cc % 2
                            l = 4 * half + cc
                            for s in range(4):
                                h = hg * 4 + s
                                co = hgh * 256 + cc * 64
                                _mm(nc, po1[s * 32:s * 32 + 32, co:co + 64],
                                    vpA[:, cc, h * 32:h * 32 + 32],
                                    scm[:, jj * 256 + s * 64:jj * 256 + s * 64 + 64],
                                    start=True, stop=False, tp=(0, s * 32))
                                _mm(nc, po1[s * 32:s * 32 + 32, co:co + 64],
                                    kvp_sb[:, l, b * 512 + h * 32:b * 512 + h * 32 + 32],
                                    qf16[:, hg, cc * 64:cc * 64 + 64],
                                    start=False, stop=True, tp=(0, s * 32))
                                _mm(nc, po2[s * 32:s * 32 + 32, co:co + 64],
                                    vpB[:, cc, h * 32:h * 32 + 32],
                                    scml[:, jj * 256 + s * 64:jj * 256 + s * 64 + 64],
                                    start=True, stop=True, tp=(0, s * 32))
                    nc.vector.tensor_copy(o1_16[:, hp, :], po1[:])
                    nc.vector.tensor_copy(o2_16[:, hp, :], po2[:])

                # ---- stats (column sums via ones-column matmuls) ----
                sq1 = [psc.tile([128, 512], F32R, tag="sq1", name=f"sq1_{rt}_{i}") for i in range(2)]
                sq2 = [psc.tile([128, 512], F32R, tag="sq2", name=f"sq2_{rt}_{i}") for i in range(2)]
                for hp in range(2):
                    nc.scalar.activation(sq1[hp][:], o1_16[:, hp, :], AF.Square)
                    nc.scalar.activation(sq2[hp][:], o2_16[:, hp, :], AF.Square)
                stA = ppst.tile([1, 512], F32, tag="stA", name=f"stA_{rt}")
                stB = ppst.tile([1, 512], F32, tag="stB", name=f"stB_{rt}")
                for g in range(4):
                    hp, gh = g // 2, g % 2
                    sl = slice(gh * 256, gh * 256 + 256)
                    _mm(nc, stA[0:1, 0:256], onc_sb[:],
                        o1_16[:, hp, sl], start=(g == 0), stop=(g == 3), skip=True)
                for g in range(4):
                    hp, gh = g // 2, g % 2
                    sl = slice(gh * 256, gh * 256 + 256)
                    _mm(nc, stA[0:1, 256:512], onc32_sb[:],
                        sq1[hp][:, sl], start=(g == 0), stop=(g == 3), skip=True)
                for g in range(4):
                    hp, gh = g // 2, g % 2
                    sl = slice(gh * 256, gh * 256 + 256)
                    _mm(nc, stB[0:1, 0:256], onc32_sb[:],
                        sq2[hp][:, sl], start=(g == 0), stop=(g == 3), skip=True)

                rows = prow.tile([1, 1792], F32, tag="rows", name=f"rows_{rt}")
                mu = rows[0:1, 0:256]
                msq = rows[0:1, 256:512]
                var = rows[0:1, 512:768]
                rstd = rows[0:1, 768:1024]
                inv2 = rows[0:1, 1024:1280]
                nc.scalar.activation(mu, stA[0:1, 0:256], AF.Copy, scale=1.0 / D)
                nc.scalar.activation(msq, stA[0:1, 256:512], AF.Copy, scale=1.0 / D)
                nc.vector.scalar_tensor_tensor(var, mu, 0.0, mu,
                                               op0=ALU.bypass, op1=ALU.mult)
                nc.vector.tensor_tensor(var, msq, var, op=ALU.subtract)
                sq_a = rows[0:1, 1280:1536]
                sq_b = rows[0:1, 1536:1792]
                nc.scalar.activation(sq_a, var, AF.Sqrt, bias=epsA[0:1, 0:1], scale=1.0)
                nc.vector.reciprocal(rstd, sq_a)
                nc.scalar.activation(sq_b, stB[0:1, 0:256], AF.Sqrt,
                                     bias=epsB[0:1, 0:1], scale=1.0 / D)
                nc.vector.reciprocal(inv2, sq_b)

                # ---- normalize (broadcast rows via K=1 ones x row matmuls) ----
                rows16 = prow.tile([1, 768], F16, tag="rows16", name=f"rows16_{rt}")
                nc.scalar.activation(rows16[0:1, 0:256], mu, AF.Copy)
                nc.scalar.activation(rows16[0:1, 256:512], rstd, AF.Copy)
                nc.scalar.activation(rows16[0:1, 512:768], inv2, AF.Copy)
                bc1 = ppo.tile([128, 512], F32, tag="po1", name=f"bc1_{rt}")
                _mm(nc, bc1[:, 0:256], on_sb[0:1, 0:128],
                    rows16[0:1, 0:256], start=True, stop=True)
                _mm(nc, bc1[:, 256:512], on_sb[0:1, 0:128],
                    rows16[0:1, 256:512], start=True, stop=True)
                bc2 = ppo.tile([128, 512], F32, tag="po2", name=f"bc2_{rt}")
                _mm(nc, bc2[:, 0:256], on_sb[0:1, 0:128],
                    rows16[0:1, 512:768], start=True, stop=True)
                for g in range(4):
                    hp, gh = g // 2, g % 2
                    sl = slice(gh * 256, gh * 256 + 256)
                    t16 = psc.tile([128, 256], F16, tag="t16",
                                   name=f"t16_{rt}_{g}")
                    nc.vector.tensor_tensor(t16[:], o1_16[:, hp, sl],
                                            bc1[:, 0:256], op=ALU.subtract)
                    nc.vector.tensor_tensor(xh1[:, hp, sl], t16[:],
                                            bc1[:, 256:512], op=ALU.mult)
                    sg16 = psc.tile([128, 256], F16, tag="sg16",
                                    name=f"sg16_{rt}_{g}")
                    nc.scalar.activation(sg16[:], o2_16[:, hp, sl], AF.Sigmoid,
                                         scale=gate_sb[:, g:g + 1])
                    u16 = psc.tile([128, 256], F16, tag="u16",
                                   name=f"u16_{rt}_{g}")
                    nc.vector.tensor_tensor(u16[:], o2_16[:, hp, sl], sg16[:],
                                            op=ALU.mult)
                    nc.vector.tensor_tensor(xh2[:, hp, sl], u16[:],
                                            bc2[:, 0:256], op=ALU.mult)

                # ---- out projections ----
                for ep in range(4):
                    pu = ppj.tile([128, 512], F32, tag="pj", name=f"pu_{rt}_{ep}")
                    for eh in range(2):
                        et = ep * 2 + eh
                        reg = pu[:, eh * 256:eh * 256 + 256]
                        for g in range(4):
                            hp, gh = g // 2, g % 2
                            sl = slice(gh * 256, gh * 256 + 256)
                            _mm(nc, reg, wo1_sb[:, g, et * 128:et * 128 + 128],
                                xh1[:, hp, sl], start=(g == 0), stop=False)
                        for g in range(4):
                            hp, gh = g // 2, g % 2
                            sl = slice(gh * 256, gh * 256 + 256)
                            _mm(nc, reg, wo2_sb[:, g, et * 128:et * 128 + 128],
                                xh2[:, hp, sl], start=False, stop=False)
                        _mm(nc, reg, ob_sb[0:1, et * 128:et * 128 + 128],
                            on_sb[:], start=False, stop=True, skip=True)
                    ot32 = po.tile([128, 512], F32, tag="ot", name=f"ot_{rt}_{ep}")
                    nc.vector.tensor_copy(ot32[:], pu[:])
                    for eh in range(2):
                        et = ep * 2 + eh
                        nc.sync.dma_start(
                            out=outT[et * 128:et * 128 + 128, r0:r0 + RT],
                            in_=ot32[:, eh * 256:eh * 256 + 256])
    nc.compile()
    return nc


def _get(name):
    if name not in _CACHE:
        _CACHE[name] = _build_launch1() if name == "l1" else _build_launch2()
    return _CACHE[name]


def kernel(**inputs):
    inp = {k: np.asarray(v, np.float32) for k, v in inputs.items()}
    q = inp["query"]                                     # (N, B, E)
    xT_all = np.ascontiguousarray(q.transpose(2, 1, 0))  # (E, B, N)
    xT_cores = [
        np.ascontiguousarray(xT_all[:, :, s * SEG:(s + 1) * SEG].reshape(E, R))
        for s in range(NCORES)
    ]
    wT = {f"{p}_{t}": np.ascontiguousarray(inp[f"w{p}_{t}"].T)
          for t in ("lin", "loc") for p in ("q", "k", "v")}
    g, bln = inp["ln_g"], inp["ln_b"]
    wo1 = 0.5 * (inp["wo_lin"] * g[None, :])
    bias1 = 0.5 * (inp["wo_lin"] @ bln + inp["bo_lin"])
    wo2 = 0.5 * (inp["wo_loc"] * inp["grn_scale"][None, :])
    bias2 = 0.5 * inp["bo_loc"]
    obias = (bias1 + bias2).astype(np.float16)[None, :]

    ones128 = np.ones((1, 128), np.float16)
    ones256 = np.ones((1, 256), np.float16)
    ident = np.eye(128, dtype=np.float16)
    mask = np.tile(np.triu(np.ones((C, C), np.float32)), (2, 8))

    # ---- launch 1: k/v projections for both branches + chunk kv states ----
    nc1 = _get("l1")
    in1 = [{
        "xT": xT_cores[s],
        "wTk": wT["k_lin"], "wTv": wT["v_lin"],
        "wTkl": wT["k_loc"], "wTvl": wT["v_loc"],
        "bk": inp["bk_lin"][:, None].astype(np.float32),
        "bkl": inp["bk_loc"][:, None].astype(np.float32),
        "bv16": inp["bv_lin"][None, :].astype(np.float16),
        "bvl16": inp["bv_loc"][None, :].astype(np.float16),
        "ones16": ones128, "ident16": ident,
    } for s in range(NCORES)]
    LAST_EXEC_NS.clear()
    r1 = run_bass_kernel_spmd(nc1, in1, list(range(NCORES)), trace=TRACE)
    res1 = r1.results
    if TRACE:
        LAST_EXEC_NS.append(r1.exec_time_ns)

    # ---- host: decode kv blocks, exclusive prefix over 64 global chunks ----
    # kvch[l, (2*(h%2)+(l%2))*32 + d, b*256 + (h//2)*32 + e] = kv[b,h,g,d,e]
    kv_all = np.zeros((NCORES * LSEG, B, H, 32, 32), np.float32)
    for s in range(NCORES):
        kvch = res1[s]["kvch"]                  # (LSEG, 128, 1024)
        for l in range(LSEG):
            for h in range(H):
                colp = (2 * (h % 2) + (l % 2)) * 32
                blk = kvch[l, colp:colp + 32].reshape(32, 4, 8, 32)
                kv_all[s * LSEG + l, :, h] = blk[:, :, h // 2, :].transpose(1, 0, 2)
    cum = np.cumsum(kv_all, axis=0)
    kvp_all = np.concatenate(
        [np.zeros((1, B, H, 32, 32), np.float32), cum[:-1]], axis=0)

    # padded per-core containers for launch 2
    def make_kpad(kfm):
        kp = np.zeros((128, H, R), np.float16)
        for h in range(H):
            kp[(h % 4) * 32:(h % 4) * 32 + 32, h, :] = kfm[h * 32:(h + 1) * 32, :]
        return kp

    def make_vpad(vtm):
        vp = np.zeros((128, B * LSEG, 512), np.float16)
        for b2 in range(B):
            for l in range(LSEG):
                vp[(l % 2) * 64:(l % 2) * 64 + 64, b2 * LSEG + l, :] = (
                    vtm[b2 * SEG + l * C:b2 * SEG + l * C + C, :])
        return vp

    def make_kvppad(kvp_seg):
        kp = np.zeros((128, LSEG, 2048), np.float16)
        for b2 in range(B):
            for h in range(H):
                kp[(h % 4) * 32:(h % 4) * 32 + 32, :,
                   b2 * 512 + h * 32:b2 * 512 + h * 32 + 32] = (
                    kvp_seg[:, b2, h].transpose(1, 0, 2))
        return kp

    # ---- launch 2 ----
    nc2 = _get("l2")
    in2 = [{
        "xT": xT_cores[s],
        "kpadA": make_kpad(res1[s]["kfm"]),
        "kpadB": make_kpad(res1[s]["klfm"]),
        "vpadA": make_vpad(res1[s]["vtm"]),
        "vpadB": make_vpad(res1[s]["vltm"]),
        "kvppad": make_kvppad(kvp_all[s * LSEG:(s + 1) * LSEG]),
        "wTq": wT["q_lin"], "wTql": wT["q_loc"],
        "bq": inp["bq_lin"][:, None].astype(np.float32),
        "bql": inp["bq_loc"][:, None].astype(np.float32),
        "wo1": np.ascontiguousarray(wo1.T).astype(np.float16),
        "wo2": np.ascontiguousarray(wo2.T).astype(np.float16),
        "ob16": obias,
        "gate": inp["grn_gate"][:, None].astype(np.float32),
        "mask": mask, "ones16": ones256,
        "onc16": np.ones((128, 1), np.float16),
        "onc32": np.ones((128, 1), np.float32),
    } for s in range(NCORES)]
    r2 = run_bass_kernel_spmd(nc2, in2, list(range(NCORES)), trace=TRACE)
    res2 = r2.results
    if TRACE:
        LAST_EXEC_NS.append(r2.exec_time_ns)

    full = np.zeros((N, B, E), np.float32)
    for s in range(NCORES):
        o = res2[s]["outT"].reshape(E, B, SEG)
        full[s * SEG:(s + 1) * SEG] = o.transpose(2, 1, 0)
    return full
